# revision 16
# baseline (speedup 1.0000x reference)
"""LIF neuron (leaky integrate-and-fire) Bass kernel for Trainium2.

Reference semantics (per element, recurrence over time axis T=32):
    mem_t   = tau * mem_{t-1} + x_t
    spike_t = 1.0 if mem_t > vth else 0.0
    mem_t   = mem_t * (1 - spike_t)        # hard reset

Input  x: [16, 32, 65536] f32  ->  Output spikes: [16, 32, 65536] f32.

Sharding: pure data parallel over batch. 8 cores x 2 batch rows each.
Per core each timestep is a [128, 1024] f32 tile (2 batches x 512
d-elements per partition).

Device writes spikes as uint8 (4x less store traffic than f32); the
host expands to f32. Spikes are exact {0,1} so this is lossless.

Per step the [128, 1024] free range is split at A between DVE and Pool,
each running the recurrence independently (two separate chains, no
cross-engine hops):
  DVE  scalar_tensor_tensor: acc[:A]  = (mem * tau) + x_t
  Pool scalar_tensor_tensor: acc[A:]  = (mem * tau) + x_t
  ACT  activation:           spk      = sigmoid(2^30*acc - 2^29) -> u8
       (2^30*(acc-vth) is sign-exact via Sterbenz and |arg|>=64 for any
        acc != vth, deep in the sigmoid clamp -> exact 0/1)
  DVE/Pool scalar_tensor_tensor: mem' = (acc <= vth) * acc  (hard reset)
DMA: 4-step groups, 1 MiB loads per batch row on the sync ring; one
4D uint8 store per group (256 KiB) issued from ACT right after the
group's last sigmoid (zero wait on ACT).

Engine budgets per core (cost model): DMA in 46.6us + out 11.7us,
DVE ~43us, Pool ~40us, ACT ~39us -> DMA bound at ~58us.
"""

import os
import sys

sys.path.insert(0, "/opt/trn_rl_repo")

import numpy as np

from concourse import bacc, mybir, tile
from concourse.bass_utils import run_bass_kernel_spmd

TAU = 0.2
VTH = 0.5

B, T, D = 16, 32, 65536
N_CORES = 8
B_SH = B // N_CORES          # 2 batch rows per core
P = 128                      # SBUF partitions
SF = B_SH * D // P           # 1024 free elems per step-tile

GS = int(os.environ.get("LIF_GS", "4"))   # timesteps per DMA group
NG = T // GS                 # groups per pass
A = int(os.environ.get("LIF_A", "1024"))  # DVE reset chunk; rest via mask-chain
C = int(os.environ.get("LIF_C", "2"))     # independent DVE chains (hide dep bubble)
J = int(os.environ.get("LIF_J", "2"))     # mask-chain subchains for [A:]

SIG_SCALE = float(2**30)
SIG_BIAS = -float(2**29)     # = -VTH * SIG_SCALE

_progs = {}


def _build_program(hw_loop=None, mode="full"):
    f32 = mybir.dt.float32
    u8 = mybir.dt.uint8
    nc = bacc.Bacc(
        "TRN2",
        target_bir_lowering=False,
        debug=False,
        enable_asserts=False,
        num_devices=N_CORES,
    )
    x = nc.dram_tensor("x", [B_SH, T, D], f32, kind="ExternalInput").ap()
    # output in SBUF-native layout [g, p, (tl b f)] so each group store is
    # one contiguous [128 x 4096B] transfer; host transposes back
    out = nc.dram_tensor(
        "out", [NG, P, GS * B_SH * (D // P)], u8, kind="ExternalOutput"
    ).ap()

    # [g, p, tl, b, f]: group, partition, step-in-group, batch, free
    xr = x.rearrange("b (g tl) (p f) -> g p tl b f", tl=GS, p=P)
    outr = out

    with tile.TileContext(nc) as tc:
        with (
            tc.tile_pool(name="xt", bufs=5) as xp,
            tc.tile_pool(name="spk", bufs=3) as sp,
            tc.tile_pool(name="acc", bufs=3) as ap_,
            tc.tile_pool(name="mem", bufs=2) as mp,
            tc.tile_pool(name="const", bufs=1) as cp,
        ):
            bias = cp.tile([P, 1], f32)
            nc.gpsimd.memset(bias[:], SIG_BIAS)
            nbias = cp.tile([P, 1], f32, tag="nbias")
            nc.gpsimd.memset(nbias[:], -SIG_BIAS)

            def body():
                one_pass(nc, tc, xr, outr, xp, sp, ap_, mp, bias, nbias, mode)

            if hw_loop is None:
                body()
            else:
                # benchmarking only: repeat the full pass in a HW loop so
                # per-pass device time can be fit from wall-clock deltas
                with tc.For_i(0, hw_loop, 1):
                    body()
    nc.compile()
    return nc


def one_pass(nc, tc, xr, outr, xp, sp, ap_, mp, bias, nbias, mode="full"):
    do_dma = mode in ("full", "dma")
    do_compute = mode in ("full", "compute")
    f32 = mybir.dt.float32
    u8 = mybir.dt.uint8
    mult = mybir.AluOpType.mult
    add = mybir.AluOpType.add
    is_le = mybir.AluOpType.is_le
    Sig = mybir.ActivationFunctionType.Sigmoid
    mem = None
    for g in range(NG):
        xt = xp.tile([P, GS * SF], f32)
        xt_v = xt[:].rearrange("p (tl b f) -> p tl b f", tl=GS, b=B_SH)
        if do_dma and g == 0:
            # head: per-step 256 KiB loads so step-0 compute starts
            # after ~1.5us instead of waiting on a full 1 MiB load
            for tl in range(GS):
                for b in range(B_SH):
                    nc.sync.dma_start(out=xt_v[:, tl, b], in_=xr[g][:, tl, b])
        elif do_dma:
            for b in range(B_SH):
                # 1 MiB load per batch row: [p:128][tl:4][f:512]
                nc.sync.dma_start(out=xt_v[:, :, b], in_=xr[g][:, :, b])
        else:
            # compute-only microbench: fabricate xt on the idle sync... use
            # gpsimd memset once per group (cheap, skews Pool slightly)
            nc.gpsimd.memset(xt[:], 0.125)
        spk = sp.tile([P, GS * SF], u8, tag="spk")
        if not do_compute:
            # dma-only microbench: keep a load->store dependency chain
            nc.gpsimd.memset(spk[:], 1)
        for tl in range(GS):
            t = g * GS + tl
            xs_lo = tl * SF
            if not do_compute:
                continue
            # DVE work [0:A] is split into C independent f-chunks whose
            # instructions interleave, hiding the per-instruction dependency
            # bubble of a single serial chain.  [A:] (if any) uses the
            # ACT-sigmoid-mask + Pool-multiply reset instead.
            cb = [(A * c // C, A * (c + 1) // C) for c in range(C)]
            jb = [
                (A + (SF - A) * j // J, A + (SF - A) * (j + 1) // J)
                for j in range(J)
            ] if A < SF else []
            if t == 0:
                # mem_{-1} = 0 -> acc = x_0 (read straight from xt)
                acct = None
                acc_full = xt[:, xs_lo : xs_lo + SF]
                def asl(lo, hi):
                    return xt[:, xs_lo + lo : xs_lo + hi]
            else:
                acct = ap_.tile([P, SF], f32)
                # acc = (mem * tau) + x_t
                for lo, hi in cb:
                    nc.vector.scalar_tensor_tensor(
                        out=acct[:, lo:hi], in0=mem[:, lo:hi], scalar=TAU,
                        in1=xt[:, xs_lo + lo : xs_lo + hi], op0=mult, op1=add,
                    )
                for lo, hi in jb:
                    nc.vector.scalar_tensor_tensor(
                        out=acct[:, lo:hi], in0=mem[:, lo:hi], scalar=TAU,
                        in1=xt[:, xs_lo + lo : xs_lo + hi], op0=mult, op1=add,
                    )
                acc_full = acct[:]
                def asl(lo, hi, acct=acct):
                    return acct[:, lo:hi]
            mem = mp.tile([P, SF], f32)
            if A < SF:
                # mask-chain reset for [A:] (J subchains, emitted before the
                # spike so ACT services the chain-critical masks first):
                # m01 = sigmoid(-2^30(acc-vth)) on ACT (exact {0,1}),
                # then mem' = m01 * acc on Pool
                m01 = ap_.tile([P, SF - A], f32, tag="m01")
                for lo, hi in jb:
                    nc.scalar.activation(
                        m01[:, lo - A : hi - A], asl(lo, hi), Sig,
                        bias=nbias[:], scale=-SIG_SCALE,
                    )
                    nc.gpsimd.tensor_tensor(
                        out=mem[:, lo:hi], in0=m01[:, lo - A : hi - A],
                        in1=asl(lo, hi), op=mybir.AluOpType.mult,
                    )
            # spike = sigmoid(2^30*acc - 2^29) -> exact {0,1} as uint8
            nc.scalar.activation(
                spk[:, xs_lo : xs_lo + SF], acc_full, Sig,
                bias=bias[:], scale=SIG_SCALE,
            )
            # mem' = (acc <= vth) * acc   (hard reset)
            for lo, hi in cb:
                nc.vector.scalar_tensor_tensor(
                    out=mem[:, lo:hi], in0=asl(lo, hi), scalar=VTH,
                    in1=asl(lo, hi), op0=is_le, op1=mult,
                )
        if do_dma:
            # one contiguous 512 KiB uint8 store per group (128 x 4KB
            # descriptors), issued from ACT right after the group's last
            # sigmoid lands (no wait on ACT)
            nc.scalar.dma_start(out=outr[g], in_=spk[:])


def _get_program(hw_loop=None, mode="full"):
    key = (hw_loop, mode)
    if key not in _progs:
        _progs[key] = _build_program(hw_loop, mode)
    return _progs[key]


def _shard(x):
    return [
        {"x": np.ascontiguousarray(x[i * B_SH : (i + 1) * B_SH])}
        for i in range(N_CORES)
    ]


def _unshard_one(arr):
    """Device out [NG, P, GS*B_SH*F] u8 -> [B_SH, T, D] f32 spikes."""
    F_ = D // P
    a = np.asarray(arr).reshape(NG, P, GS, B_SH, F_)
    a = np.transpose(a, (3, 0, 2, 1, 4))  # [b, g, tl, p, f]
    return a.reshape(B_SH, T, D).astype(np.float32)


def kernel(x):
    x = np.asarray(x, dtype=np.float32)
    assert x.shape == (B, T, D), x.shape
    nc = _get_program()
    res = run_bass_kernel_spmd(nc, _shard(x), list(range(N_CORES)))
    return np.concatenate(
        [_unshard_one(res.results[i]["out"]) for i in range(N_CORES)], axis=0
    )


# revision 20
# speedup vs baseline: 1.0792x; 1.0792x over previous
"""LIF neuron (leaky integrate-and-fire) Bass kernel for Trainium2.

Reference semantics (per element, recurrence over time axis T=32):
    mem_t   = tau * mem_{t-1} + x_t
    spike_t = 1.0 if mem_t > vth else 0.0
    mem_t   = mem_t * (1 - spike_t)        # hard reset

Input  x: [16, 32, 65536] f32  ->  Output spikes: [16, 32, 65536] f32.

Sharding: pure data parallel over batch. 8 cores x 2 batch rows each.
Per core each timestep is a [128, 1024] f32 tile (2 batches x 512
d-elements per partition).

Device writes spikes as uint8 (4x less store traffic than f32); the
host expands to f32. Spikes are exact {0,1} so this is lossless.

Per step the [128, 1024] free range is split at A between DVE and Pool,
each running the recurrence independently (two separate chains, no
cross-engine hops):
  DVE  scalar_tensor_tensor: acc[:A]  = (mem * tau) + x_t
  Pool scalar_tensor_tensor: acc[A:]  = (mem * tau) + x_t
  ACT  activation:           spk      = sigmoid(2^30*acc - 2^29) -> u8
       (2^30*(acc-vth) is sign-exact via Sterbenz and |arg|>=64 for any
        acc != vth, deep in the sigmoid clamp -> exact 0/1)
  DVE/Pool scalar_tensor_tensor: mem' = (acc <= vth) * acc  (hard reset)
DMA: 4-step groups, 1 MiB loads per batch row on the sync ring; one
4D uint8 store per group (256 KiB) issued from ACT right after the
group's last sigmoid (zero wait on ACT).

Engine budgets per core (cost model): DMA in 46.6us + out 11.7us,
DVE ~43us, Pool ~40us, ACT ~39us -> DMA bound at ~58us.
"""

import os
import sys

sys.path.insert(0, "/opt/trn_rl_repo")

import numpy as np

from concourse import bacc, mybir, tile
from concourse.bass_utils import run_bass_kernel_spmd

TAU = 0.2
VTH = 0.5

B, T, D = 16, 32, 65536
N_CORES = 8
B_SH = B // N_CORES          # 2 batch rows per core
P = 128                      # SBUF partitions
SF = B_SH * D // P           # 1024 free elems per step-tile

GS = int(os.environ.get("LIF_GS", "4"))   # timesteps per DMA group
NG = T // GS                 # groups per pass
A = int(os.environ.get("LIF_A", "1024"))  # DVE reset chunk; rest via mask-chain
C = int(os.environ.get("LIF_C", "2"))     # independent DVE chains (hide dep bubble)
J = int(os.environ.get("LIF_J", "2"))     # mask-chain subchains for [A:]

SIG_SCALE = float(2**30)
SIG_BIAS = -float(2**29)     # = -VTH * SIG_SCALE

_progs = {}


def _build_program(hw_loop=None, mode="full"):
    f32 = mybir.dt.float32
    u8 = mybir.dt.uint8
    nc = bacc.Bacc(
        "TRN2",
        target_bir_lowering=False,
        debug=False,
        enable_asserts=False,
        num_devices=N_CORES,
    )
    x = nc.dram_tensor("x", [B_SH, T, D], f32, kind="ExternalInput").ap()
    # output in SBUF-native layout [g, p, (tl b f)] so each group store is
    # one contiguous [128 x 4096B] transfer; host transposes back
    out = nc.dram_tensor(
        "out", [NG, P, GS * B_SH * (D // P)], u8, kind="ExternalOutput"
    ).ap()

    # [g, p, tl, b, f]: group, partition, step-in-group, batch, free
    xr = x.rearrange("b (g tl) (p f) -> g p tl b f", tl=GS, p=P)
    outr = out

    with tile.TileContext(nc) as tc:
        with (
            tc.tile_pool(name="xt", bufs=5) as xp,
            tc.tile_pool(name="spk", bufs=3) as sp,
            tc.tile_pool(name="acc", bufs=3) as ap_,
            tc.tile_pool(name="m01", bufs=3) as mq,
            tc.tile_pool(name="mem", bufs=2) as mp,
            tc.tile_pool(name="const", bufs=1) as cp,
        ):
            bias = cp.tile([P, 1], f32)
            nc.gpsimd.memset(bias[:], SIG_BIAS)
            nbias = cp.tile([P, 1], f32, tag="nbias")
            nc.gpsimd.memset(nbias[:], -SIG_BIAS)

            def body():
                one_pass(nc, tc, xr, outr, xp, sp, ap_, mq, mp, bias, nbias, mode)

            if hw_loop is None:
                body()
            else:
                # benchmarking only: repeat the full pass in a HW loop so
                # per-pass device time can be fit from wall-clock deltas
                with tc.For_i(0, hw_loop, 1):
                    body()
    nc.compile()
    return nc


def one_pass(nc, tc, xr, outr, xp, sp, ap_, mq, mp, bias, nbias, mode="full"):
    do_dma = mode in ("full", "dma")
    do_compute = mode in ("full", "compute")
    f32 = mybir.dt.float32
    u8 = mybir.dt.uint8
    mult = mybir.AluOpType.mult
    add = mybir.AluOpType.add
    is_le = mybir.AluOpType.is_le
    Sig = mybir.ActivationFunctionType.Sigmoid
    mem = None
    for g in range(NG):
        xt = xp.tile([P, GS * SF], f32)
        xt_v = xt[:].rearrange("p (tl b f) -> p tl b f", tl=GS, b=B_SH)
        if do_dma and g == 0:
            # head: per-step 256 KiB loads so step-0 compute starts
            # after ~1.5us instead of waiting on a full 1 MiB load
            for tl in range(GS):
                for b in range(B_SH):
                    nc.sync.dma_start(out=xt_v[:, tl, b], in_=xr[g][:, tl, b])
        elif do_dma:
            for b in range(B_SH):
                # 1 MiB load per batch row: [p:128][tl:4][f:512]
                nc.sync.dma_start(out=xt_v[:, :, b], in_=xr[g][:, :, b])
        else:
            # compute-only microbench: fabricate xt on the idle sync... use
            # gpsimd memset once per group (cheap, skews Pool slightly)
            nc.gpsimd.memset(xt[:], 0.125)
        spk = sp.tile([P, GS * SF], u8, tag="spk")
        if not do_compute:
            # dma-only microbench: keep a load->store dependency chain
            nc.gpsimd.memset(spk[:], 1)
        for tl in range(GS):
            t = g * GS + tl
            xs_lo = tl * SF
            if not do_compute:
                continue
            # DVE work [0:A] is split into C independent f-chunks whose
            # instructions interleave, hiding the per-instruction dependency
            # bubble of a single serial chain.  [A:] (if any) uses the
            # ACT-sigmoid-mask + Pool-multiply reset instead.
            cb = [(A * c // C, A * (c + 1) // C) for c in range(C)]
            jb = [
                (A + (SF - A) * j // J, A + (SF - A) * (j + 1) // J)
                for j in range(J)
            ] if A < SF else []
            if t == 0:
                # mem_{-1} = 0 -> acc = x_0 (read straight from xt)
                acct = None
                acc_full = xt[:, xs_lo : xs_lo + SF]
                def asl(lo, hi):
                    return xt[:, xs_lo + lo : xs_lo + hi]
            else:
                acct = ap_.tile([P, SF], f32)
                # acc = (mem * tau) + x_t   ([0:A] chunks first)
                for lo, hi in cb:
                    nc.vector.scalar_tensor_tensor(
                        out=acct[:, lo:hi], in0=mem[:, lo:hi], scalar=TAU,
                        in1=xt[:, xs_lo + lo : xs_lo + hi], op0=mult, op1=add,
                    )
                acc_full = acct[:]
                def asl(lo, hi, acct=acct):
                    return acct[:, lo:hi]
            prev_mem = mem
            mem = mp.tile([P, SF], f32)
            if A < SF:
                m01 = mq.tile([P, SF - A], f32, tag="m01")
            else:
                m01 = None
            # Interleave on DVE: [A:] mask-chain accs between the [0:A]
            # resets so dependent DVE pairs are never back-to-back even at
            # C == 1.  Per [A:] subchain j: acc (DVE) -> m01 = sigmoid(
            # -2^30(acc-vth)) (ACT, exact {0,1}) -> mem' = m01*acc (Pool).
            for i in range(max(J if A < SF else 0, C)):
                if A < SF and i < J:
                    lo, hi = jb[i]
                    if t > 0:
                        nc.vector.scalar_tensor_tensor(
                            out=acct[:, lo:hi], in0=prev_mem[:, lo:hi],
                            scalar=TAU, in1=xt[:, xs_lo + lo : xs_lo + hi],
                            op0=mult, op1=add,
                        )
                    nc.scalar.activation(
                        m01[:, lo - A : hi - A], asl(lo, hi), Sig,
                        bias=nbias[:], scale=-SIG_SCALE,
                    )
                    nc.gpsimd.tensor_tensor(
                        out=mem[:, lo:hi], in0=m01[:, lo - A : hi - A],
                        in1=asl(lo, hi), op=mybir.AluOpType.mult,
                    )
                if i < C:
                    lo, hi = cb[i]
                    # mem' = (acc <= vth) * acc   (hard reset)
                    nc.vector.scalar_tensor_tensor(
                        out=mem[:, lo:hi], in0=asl(lo, hi), scalar=VTH,
                        in1=asl(lo, hi), op0=is_le, op1=mult,
                    )
            # spike = sigmoid(2^30*acc - 2^29) -> exact {0,1} as uint8
            nc.scalar.activation(
                spk[:, xs_lo : xs_lo + SF], acc_full, Sig,
                bias=bias[:], scale=SIG_SCALE,
            )
        if do_dma:
            # one contiguous 512 KiB uint8 store per group (128 x 4KB
            # descriptors), issued from ACT right after the group's last
            # sigmoid lands (no wait on ACT)
            nc.scalar.dma_start(out=outr[g], in_=spk[:])


def _get_program(hw_loop=None, mode="full"):
    key = (hw_loop, mode)
    if key not in _progs:
        _progs[key] = _build_program(hw_loop, mode)
    return _progs[key]


def _shard(x):
    return [
        {"x": np.ascontiguousarray(x[i * B_SH : (i + 1) * B_SH])}
        for i in range(N_CORES)
    ]


def _unshard_one(arr):
    """Device out [NG, P, GS*B_SH*F] u8 -> [B_SH, T, D] f32 spikes."""
    F_ = D // P
    a = np.asarray(arr).reshape(NG, P, GS, B_SH, F_)
    a = np.transpose(a, (3, 0, 2, 1, 4))  # [b, g, tl, p, f]
    return a.reshape(B_SH, T, D).astype(np.float32)


def kernel(x):
    x = np.asarray(x, dtype=np.float32)
    assert x.shape == (B, T, D), x.shape
    nc = _get_program()
    res = run_bass_kernel_spmd(nc, _shard(x), list(range(N_CORES)))
    return np.concatenate(
        [_unshard_one(res.results[i]["out"]) for i in range(N_CORES)], axis=0
    )


# revision 21
# speedup vs baseline: 1.2328x; 1.1424x over previous
"""LIF neuron (leaky integrate-and-fire) Bass kernel for Trainium2.

Reference semantics (per element, recurrence over time axis T=32):
    mem_t   = tau * mem_{t-1} + x_t
    spike_t = 1.0 if mem_t > vth else 0.0
    mem_t   = mem_t * (1 - spike_t)        # hard reset

Input  x: [16, 32, 65536] f32  ->  Output spikes: [16, 32, 65536] f32.

Sharding: pure data parallel over batch. 8 cores x 2 batch rows each.
Per core each timestep is a [128, 1024] f32 tile (2 batches x 512
d-elements per partition).

Device writes spikes as uint8 (4x less store traffic than f32); the
host expands to f32. Spikes are exact {0,1} so this is lossless.

Per step the [128, 1024] free range is split at A between DVE and Pool,
each running the recurrence independently (two separate chains, no
cross-engine hops):
  DVE  scalar_tensor_tensor: acc[:A]  = (mem * tau) + x_t
  Pool scalar_tensor_tensor: acc[A:]  = (mem * tau) + x_t
  ACT  activation:           spk      = sigmoid(2^30*acc - 2^29) -> u8
       (2^30*(acc-vth) is sign-exact via Sterbenz and |arg|>=64 for any
        acc != vth, deep in the sigmoid clamp -> exact 0/1)
  DVE/Pool scalar_tensor_tensor: mem' = (acc <= vth) * acc  (hard reset)
DMA: 4-step groups, 1 MiB loads per batch row on the sync ring; one
4D uint8 store per group (256 KiB) issued from ACT right after the
group's last sigmoid (zero wait on ACT).

Engine budgets per core (cost model): DMA in 46.6us + out 11.7us,
DVE ~43us, Pool ~40us, ACT ~39us -> DMA bound at ~58us.
"""

import os
import sys

sys.path.insert(0, "/opt/trn_rl_repo")

import numpy as np

from concourse import bacc, mybir, tile
from concourse.bass_utils import run_bass_kernel_spmd

TAU = 0.2
VTH = 0.5

B, T, D = 16, 32, 65536
N_CORES = 8
B_SH = B // N_CORES          # 2 batch rows per core
P = 128                      # SBUF partitions
SF = B_SH * D // P           # 1024 free elems per step-tile

GS = int(os.environ.get("LIF_GS", "4"))   # timesteps per DMA group
NG = T // GS                 # groups per pass
A = int(os.environ.get("LIF_A", "1024"))  # DVE reset chunk; rest via mask-chain
C = int(os.environ.get("LIF_C", "2"))     # independent DVE chains (hide dep bubble)
J = int(os.environ.get("LIF_J", "2"))     # mask-chain subchains for [A:]

SIG_SCALE = float(2**30)
SIG_BIAS = -float(2**29)     # = -VTH * SIG_SCALE

_progs = {}


def _build_program(hw_loop=None, mode="full"):
    f32 = mybir.dt.float32
    u8 = mybir.dt.uint8
    nc = bacc.Bacc(
        "TRN2",
        target_bir_lowering=False,
        debug=False,
        enable_asserts=False,
        num_devices=N_CORES,
    )
    x = nc.dram_tensor("x", [B_SH, T, D], f32, kind="ExternalInput").ap()
    # output in SBUF-native layout [g, p, (tl b f)] so each group store is
    # one contiguous [128 x 4096B] transfer; host transposes back
    out = nc.dram_tensor(
        "out", [NG, P, GS * B_SH * (D // P)], u8, kind="ExternalOutput"
    ).ap()

    # [g, p, tl, b, f]: group, partition, step-in-group, batch, free
    xr = x.rearrange("b (g tl) (p f) -> g p tl b f", tl=GS, p=P)
    outr = out

    with tile.TileContext(nc) as tc:
        xt_bufs = {2: 8, 4: 5, 8: 4}[GS]
        with (
            tc.tile_pool(name="xt", bufs=xt_bufs) as xp,
            tc.tile_pool(name="spk", bufs=3) as sp,
            tc.tile_pool(name="acc", bufs=4) as ap_,
            tc.tile_pool(name="m01", bufs=3) as mq,
            tc.tile_pool(name="mem", bufs=3) as mp,
            tc.tile_pool(name="const", bufs=1) as cp,
        ):
            bias = cp.tile([P, 1], f32)
            nc.gpsimd.memset(bias[:], SIG_BIAS)
            nbias = cp.tile([P, 1], f32, tag="nbias")
            nc.gpsimd.memset(nbias[:], -SIG_BIAS)

            def body():
                one_pass(nc, tc, xr, outr, xp, sp, ap_, mq, mp, bias, nbias, mode)

            if hw_loop is None:
                body()
            else:
                # benchmarking only: repeat the full pass in a HW loop so
                # per-pass device time can be fit from wall-clock deltas
                with tc.For_i(0, hw_loop, 1):
                    body()
    nc.compile()
    return nc


def one_pass(nc, tc, xr, outr, xp, sp, ap_, mq, mp, bias, nbias, mode="full"):
    do_dma = mode in ("full", "dma")
    do_compute = mode in ("full", "compute")
    f32 = mybir.dt.float32
    u8 = mybir.dt.uint8
    mult = mybir.AluOpType.mult
    add = mybir.AluOpType.add
    is_le = mybir.AluOpType.is_le
    Sig = mybir.ActivationFunctionType.Sigmoid
    mem = None
    for g in range(NG):
        xt = xp.tile([P, GS * SF], f32)
        xt_v = xt[:].rearrange("p (tl b f) -> p tl b f", tl=GS, b=B_SH)
        if do_dma and g == 0:
            # head: per-step 256 KiB loads so step-0 compute starts
            # after ~1.5us instead of waiting on a full 1 MiB load
            for tl in range(GS):
                for b in range(B_SH):
                    nc.sync.dma_start(out=xt_v[:, tl, b], in_=xr[g][:, tl, b])
        elif do_dma:
            for b in range(B_SH):
                # 1 MiB load per batch row: [p:128][tl:4][f:512]
                nc.sync.dma_start(out=xt_v[:, :, b], in_=xr[g][:, :, b])
        else:
            # compute-only microbench: fabricate xt on the idle sync... use
            # gpsimd memset once per group (cheap, skews Pool slightly)
            nc.gpsimd.memset(xt[:], 0.125)
        spk = sp.tile([P, GS * SF], u8, tag="spk")
        if not do_compute:
            # dma-only microbench: keep a load->store dependency chain
            nc.gpsimd.memset(spk[:], 1)
        for tl in range(GS):
            t = g * GS + tl
            xs_lo = tl * SF
            if not do_compute:
                continue
            # DVE work [0:A] is split into C independent f-chunks whose
            # instructions interleave, hiding the per-instruction dependency
            # bubble of a single serial chain.  [A:] (if any) uses the
            # ACT-sigmoid-mask + Pool-multiply reset instead.
            cb = [(A * c // C, A * (c + 1) // C) for c in range(C)]
            jb = [
                (A + (SF - A) * j // J, A + (SF - A) * (j + 1) // J)
                for j in range(J)
            ] if A < SF else []
            if t == 0:
                # mem_{-1} = 0 -> acc = x_0 (read straight from xt)
                acct = None
                acc_full = xt[:, xs_lo : xs_lo + SF]
                def asl(lo, hi):
                    return xt[:, xs_lo + lo : xs_lo + hi]
            else:
                acct = ap_.tile([P, SF], f32)
                # acc = (mem * tau) + x_t   ([0:A] chunks first)
                for lo, hi in cb:
                    nc.vector.scalar_tensor_tensor(
                        out=acct[:, lo:hi], in0=mem[:, lo:hi], scalar=TAU,
                        in1=xt[:, xs_lo + lo : xs_lo + hi], op0=mult, op1=add,
                    )
                acc_full = acct[:]
                def asl(lo, hi, acct=acct):
                    return acct[:, lo:hi]
            prev_mem = mem
            mem = mp.tile([P, SF], f32)
            if A < SF:
                m01 = mq.tile([P, SF - A], f32, tag="m01")
            else:
                m01 = None
            # Interleave on DVE: [A:] mask-chain accs between the [0:A]
            # resets so dependent DVE pairs are never back-to-back even at
            # C == 1.  Per [A:] subchain j: acc (DVE) -> m01 = sigmoid(
            # -2^30(acc-vth)) (ACT, exact {0,1}) -> mem' = m01*acc (Pool).
            for i in range(max(J if A < SF else 0, C)):
                if A < SF and i < J:
                    lo, hi = jb[i]
                    if t > 0:
                        nc.vector.scalar_tensor_tensor(
                            out=acct[:, lo:hi], in0=prev_mem[:, lo:hi],
                            scalar=TAU, in1=xt[:, xs_lo + lo : xs_lo + hi],
                            op0=mult, op1=add,
                        )
                    nc.scalar.activation(
                        m01[:, lo - A : hi - A], asl(lo, hi), Sig,
                        bias=nbias[:], scale=-SIG_SCALE,
                    )
                    nc.gpsimd.tensor_tensor(
                        out=mem[:, lo:hi], in0=m01[:, lo - A : hi - A],
                        in1=asl(lo, hi), op=mybir.AluOpType.mult,
                    )
                if i < C:
                    lo, hi = cb[i]
                    # mem' = (acc <= vth) * acc   (hard reset)
                    nc.vector.scalar_tensor_tensor(
                        out=mem[:, lo:hi], in0=asl(lo, hi), scalar=VTH,
                        in1=asl(lo, hi), op0=is_le, op1=mult,
                    )
            # spike = sigmoid(2^30*acc - 2^29) -> exact {0,1} as uint8
            nc.scalar.activation(
                spk[:, xs_lo : xs_lo + SF], acc_full, Sig,
                bias=bias[:], scale=SIG_SCALE,
            )
        if do_dma:
            # one contiguous 512 KiB uint8 store per group (128 x 4KB
            # descriptors), issued from ACT right after the group's last
            # sigmoid lands (no wait on ACT)
            nc.scalar.dma_start(out=outr[g], in_=spk[:])


def _get_program(hw_loop=None, mode="full"):
    key = (hw_loop, mode)
    if key not in _progs:
        _progs[key] = _build_program(hw_loop, mode)
    return _progs[key]


def _shard(x):
    return [
        {"x": np.ascontiguousarray(x[i * B_SH : (i + 1) * B_SH])}
        for i in range(N_CORES)
    ]


def _unshard_one(arr):
    """Device out [NG, P, GS*B_SH*F] u8 -> [B_SH, T, D] f32 spikes."""
    F_ = D // P
    a = np.asarray(arr).reshape(NG, P, GS, B_SH, F_)
    a = np.transpose(a, (3, 0, 2, 1, 4))  # [b, g, tl, p, f]
    return a.reshape(B_SH, T, D).astype(np.float32)


def kernel(x):
    x = np.asarray(x, dtype=np.float32)
    assert x.shape == (B, T, D), x.shape
    nc = _get_program()
    res = run_bass_kernel_spmd(nc, _shard(x), list(range(N_CORES)))
    return np.concatenate(
        [_unshard_one(res.results[i]["out"]) for i in range(N_CORES)], axis=0
    )


# revision 23
# speedup vs baseline: 1.2630x; 1.0245x over previous
"""LIF neuron (leaky integrate-and-fire) Bass kernel for Trainium2.

Reference semantics (per element, recurrence over time axis T=32):
    mem_t   = tau * mem_{t-1} + x_t
    spike_t = 1.0 if mem_t > vth else 0.0
    mem_t   = mem_t * (1 - spike_t)        # hard reset

Input  x: [16, 32, 65536] f32  ->  Output spikes: [16, 32, 65536] f32.

Sharding: pure data parallel over batch. 8 cores x 2 batch rows each.
Per core each timestep is a [128, 1024] f32 tile (2 batches x 512
d-elements per partition).

Device writes spikes as uint8 (4x less store traffic than f32); the
host expands to f32. Spikes are exact {0,1} so this is lossless.

Design (measured-driven; HW exec ~86 us vs 111 us baseline):
  DVE  scalar_tensor_tensor: acc  = (mem * tau) + x_t
  ACT  activation:           spk  = sigmoid(2^30*acc - 2^29) -> u8
       (2^30*(acc-vth) is sign-exact via Sterbenz and |arg|>=64 for any
        acc != vth, deep in the sigmoid clamp -> exact 0/1; one ACT op
        replaces the baseline's Sign+Relu pair)
  DVE  scalar_tensor_tensor: mem' = (acc <= vth) * acc  (hard reset)
The recurrence is DVE-bound: each dependent back-to-back DVE pair pays
a ~550 ns bubble, so the [128, 1024] step is split into C=2 independent
f-chunks whose instructions interleave (C=2 measured optimal: 86.3 us
vs 109 us at C=1, 110 us at C=4 -- per-instr fixed cost ~140-165 ns).
Pool cannot run scalar_tensor_tensor (walrus engine check), and
cross-engine reset offload (ACT sigmoid-mask + Pool multiply, LIF_A <
1024) loses to per-step hop latency: 102 us measured at A=640.

DMA: 4-step groups, 1 MiB loads per batch row on the sync ring; the
uint8 store per group is a single contiguous [128 x 4 KiB] transfer in
SBUF-native layout (host transposes back), issued from ACT right after
the group's last sigmoid (zero wait on ACT).

Engine budgets per core: DVE 68.3 us exec + ~18 us instr overhead
(bound), ACT ~41 us, DMA in ~47 us + out ~12 us overlapped.
"""

import os
import sys

sys.path.insert(0, "/opt/trn_rl_repo")

import numpy as np

from concourse import bacc, mybir, tile
from concourse.bass_utils import run_bass_kernel_spmd

TAU = 0.2
VTH = 0.5

B, T, D = 16, 32, 65536
N_CORES = 8
B_SH = B // N_CORES          # 2 batch rows per core
P = 128                      # SBUF partitions
SF = B_SH * D // P           # 1024 free elems per step-tile

GS = int(os.environ.get("LIF_GS", "4"))   # timesteps per DMA group
NG = T // GS                 # groups per pass
A = int(os.environ.get("LIF_A", "1024"))  # DVE reset chunk; rest via mask-chain
C = int(os.environ.get("LIF_C", "2"))     # independent DVE chains (hide dep bubble)
J = int(os.environ.get("LIF_J", "2"))     # mask-chain subchains for [A:]

SIG_SCALE = float(2**30)
SIG_BIAS = -float(2**29)     # = -VTH * SIG_SCALE

_progs = {}


def _build_program(hw_loop=None, mode="full"):
    f32 = mybir.dt.float32
    u8 = mybir.dt.uint8
    nc = bacc.Bacc(
        "TRN2",
        target_bir_lowering=False,
        debug=False,
        enable_asserts=False,
        num_devices=N_CORES,
    )
    x = nc.dram_tensor("x", [B_SH, T, D], f32, kind="ExternalInput").ap()
    # output in SBUF-native layout [g, p, (tl b f)] so each group store is
    # one contiguous [128 x 4096B] transfer; host transposes back
    out = nc.dram_tensor(
        "out", [NG, P, GS * B_SH * (D // P)], u8, kind="ExternalOutput"
    ).ap()

    # [g, p, tl, b, f]: group, partition, step-in-group, batch, free
    xr = x.rearrange("b (g tl) (p f) -> g p tl b f", tl=GS, p=P)
    outr = out

    with tile.TileContext(nc) as tc:
        xt_bufs = {2: 8, 4: 5, 8: 4}[GS]
        with (
            tc.tile_pool(name="xt", bufs=xt_bufs) as xp,
            tc.tile_pool(name="spk", bufs=3) as sp,
            tc.tile_pool(name="acc", bufs=3) as ap_,
            tc.tile_pool(name="m01", bufs=3) as mq,
            tc.tile_pool(name="mem", bufs=2) as mp,
            tc.tile_pool(name="const", bufs=1) as cp,
        ):
            bias = cp.tile([P, 1], f32)
            nc.gpsimd.memset(bias[:], SIG_BIAS)
            nbias = cp.tile([P, 1], f32, tag="nbias")
            nc.gpsimd.memset(nbias[:], -SIG_BIAS)

            def body():
                one_pass(nc, tc, xr, outr, xp, sp, ap_, mq, mp, bias, nbias, mode)

            if hw_loop is None:
                body()
            else:
                # benchmarking only: repeat the full pass in a HW loop so
                # per-pass device time can be fit from wall-clock deltas
                with tc.For_i(0, hw_loop, 1):
                    body()
    nc.compile()
    return nc


def one_pass(nc, tc, xr, outr, xp, sp, ap_, mq, mp, bias, nbias, mode="full"):
    do_dma = mode in ("full", "dma")
    do_compute = mode in ("full", "compute")
    f32 = mybir.dt.float32
    u8 = mybir.dt.uint8
    mult = mybir.AluOpType.mult
    add = mybir.AluOpType.add
    is_le = mybir.AluOpType.is_le
    Sig = mybir.ActivationFunctionType.Sigmoid
    mem = None
    for g in range(NG):
        xt = xp.tile([P, GS * SF], f32)
        xt_v = xt[:].rearrange("p (tl b f) -> p tl b f", tl=GS, b=B_SH)
        if do_dma and g == 0:
            # head: per-step 256 KiB loads so step-0 compute starts
            # after ~1.5us instead of waiting on a full 1 MiB load
            for tl in range(GS):
                for b in range(B_SH):
                    nc.sync.dma_start(out=xt_v[:, tl, b], in_=xr[g][:, tl, b])
        elif do_dma:
            for b in range(B_SH):
                # 1 MiB load per batch row: [p:128][tl:4][f:512]
                nc.sync.dma_start(out=xt_v[:, :, b], in_=xr[g][:, :, b])
        else:
            # compute-only microbench: fabricate xt on the idle sync... use
            # gpsimd memset once per group (cheap, skews Pool slightly)
            nc.gpsimd.memset(xt[:], 0.125)
        spk = sp.tile([P, GS * SF], u8, tag="spk")
        if not do_compute:
            # dma-only microbench: keep a load->store dependency chain
            nc.gpsimd.memset(spk[:], 1)
        for tl in range(GS):
            t = g * GS + tl
            xs_lo = tl * SF
            if not do_compute:
                continue
            # DVE work [0:A] is split into C independent f-chunks whose
            # instructions interleave, hiding the per-instruction dependency
            # bubble of a single serial chain.  [A:] (if any) uses the
            # ACT-sigmoid-mask + Pool-multiply reset instead.
            cb = [(A * c // C, A * (c + 1) // C) for c in range(C)]
            jb = [
                (A + (SF - A) * j // J, A + (SF - A) * (j + 1) // J)
                for j in range(J)
            ] if A < SF else []
            if t == 0:
                # mem_{-1} = 0 -> acc = x_0 (read straight from xt)
                acct = None
                acc_full = xt[:, xs_lo : xs_lo + SF]
                def asl(lo, hi):
                    return xt[:, xs_lo + lo : xs_lo + hi]
            else:
                acct = ap_.tile([P, SF], f32)
                # acc = (mem * tau) + x_t   ([0:A] chunks first)
                for lo, hi in cb:
                    nc.vector.scalar_tensor_tensor(
                        out=acct[:, lo:hi], in0=mem[:, lo:hi], scalar=TAU,
                        in1=xt[:, xs_lo + lo : xs_lo + hi], op0=mult, op1=add,
                    )
                acc_full = acct[:]
                def asl(lo, hi, acct=acct):
                    return acct[:, lo:hi]
            prev_mem = mem
            mem = mp.tile([P, SF], f32)
            if A < SF:
                m01 = mq.tile([P, SF - A], f32, tag="m01")
            else:
                m01 = None
            # Interleave on DVE: [A:] mask-chain accs between the [0:A]
            # resets so dependent DVE pairs are never back-to-back even at
            # C == 1.  Per [A:] subchain j: acc (DVE) -> m01 = sigmoid(
            # -2^30(acc-vth)) (ACT, exact {0,1}) -> mem' = m01*acc (Pool).
            for i in range(max(J if A < SF else 0, C)):
                if A < SF and i < J:
                    lo, hi = jb[i]
                    if t > 0:
                        nc.vector.scalar_tensor_tensor(
                            out=acct[:, lo:hi], in0=prev_mem[:, lo:hi],
                            scalar=TAU, in1=xt[:, xs_lo + lo : xs_lo + hi],
                            op0=mult, op1=add,
                        )
                    nc.scalar.activation(
                        m01[:, lo - A : hi - A], asl(lo, hi), Sig,
                        bias=nbias[:], scale=-SIG_SCALE,
                    )
                    nc.gpsimd.tensor_tensor(
                        out=mem[:, lo:hi], in0=m01[:, lo - A : hi - A],
                        in1=asl(lo, hi), op=mybir.AluOpType.mult,
                    )
                if i < C:
                    lo, hi = cb[i]
                    # mem' = (acc <= vth) * acc   (hard reset)
                    nc.vector.scalar_tensor_tensor(
                        out=mem[:, lo:hi], in0=asl(lo, hi), scalar=VTH,
                        in1=asl(lo, hi), op0=is_le, op1=mult,
                    )
            # spike = sigmoid(2^30*acc - 2^29) -> exact {0,1} as uint8
            nc.scalar.activation(
                spk[:, xs_lo : xs_lo + SF], acc_full, Sig,
                bias=bias[:], scale=SIG_SCALE,
            )
        if do_dma:
            # one contiguous 512 KiB uint8 store per group (128 x 4KB
            # descriptors), issued from ACT right after the group's last
            # sigmoid lands (no wait on ACT)
            nc.scalar.dma_start(out=outr[g], in_=spk[:])


def _get_program(hw_loop=None, mode="full"):
    key = (hw_loop, mode)
    if key not in _progs:
        _progs[key] = _build_program(hw_loop, mode)
    return _progs[key]


def _shard(x):
    return [
        {"x": np.ascontiguousarray(x[i * B_SH : (i + 1) * B_SH])}
        for i in range(N_CORES)
    ]


def _unshard_one(arr):
    """Device out [NG, P, GS*B_SH*F] u8 -> [B_SH, T, D] f32 spikes."""
    F_ = D // P
    a = np.asarray(arr).reshape(NG, P, GS, B_SH, F_)
    a = np.transpose(a, (3, 0, 2, 1, 4))  # [b, g, tl, p, f]
    return a.reshape(B_SH, T, D).astype(np.float32)


def kernel(x):
    x = np.asarray(x, dtype=np.float32)
    assert x.shape == (B, T, D), x.shape
    nc = _get_program()
    res = run_bass_kernel_spmd(nc, _shard(x), list(range(N_CORES)))
    return np.concatenate(
        [_unshard_one(res.results[i]["out"]) for i in range(N_CORES)], axis=0
    )
